# revision 3
# baseline (speedup 1.0000x reference)
"""LocallyConnected2d Trainium2 kernel (v3).

y[b,o,l] = sum_k x_unf[b,k,l] * w[o,k,l]   (B=64, K=864, L=1024, O=192)

Strategy: shard L across 8 cores (128 locations = 4 output rows each).

Traffic: weights dominate (read once, 170M elements). Stored fp8e3
(4 mantissa bits; w is uniform so e3m4 with scale 320 gives ~1.2% rms
quant err -> ~1.1e-2 max rel output err). The 1/320 descale is folded
into x on the host. x is NOT unfolded: the raw padded slice
[96c, 6h, 34w, 64b] fp16 lives in SBUF and every matmul's stationary
operand is a shifted window of it (zero-copy unfold).

Compute: contraction ordered (kh,kw)-major -> 9 chunks of K=96 (=C_IN).
Two adjacent locations share one matmul: stationary = x window
[96, 2*64=128] (full PE M), moving = their weights [96, 2*192=384]
fp8e3 at 1 row/cycle. Off-diagonal PSUM quadrants are computed but
ignored. Adjacent pairs' 9-matmul accumulation chains are interleaved
on the tensor queue so one chain's PSUM drain hides under the other.

Output: per-pair PSUM diagonals are copied (cast fp32->fp16) into a
parity-compacted staging tile (even locations partitions 0-63, odd
64-127; vector does evens, scalar does odds) so the output DMA reads
contiguous 1.5KB partition lines.
"""

import sys

sys.path.insert(0, "/opt/trn_rl_repo")

import numpy as np
import ml_dtypes

B = 64
C_IN = 96
H = W = 32
C_OUT = 192
KS = 3
L = 1024
NCORES = 8
NL = L // NCORES      # 128 locations per core
ROWS = H // NCORES    # 4 output rows per core
NBLK = 16             # blocks of 8 locations
BL = 8                # locations per block
NP = BL // 2          # pairs per block
WSCALE = 320.0        # fp8e3 weight scale (max |w|*320 ~ 15.4 < 15.5)

_cached = None


def _build_program():
    from concourse import bacc, bass, tile, mybir

    nc = bacc.Bacc("TRN2", target_bir_lowering=False, debug=False,
                   num_devices=NCORES)
    # x split in two 3-row chunks so compute can start after chunk 0
    xs_d = nc.dram_tensor("xs", [2, C_IN, 3, W + 2, B], mybir.dt.float16,
                          kind="ExternalInput")
    w_d = nc.dram_tensor("w", [NBLK, C_IN, BL, KS * KS, C_OUT],
                         mybir.dt.float8e3, kind="ExternalInput")
    # y[b, parity, l//2, o]; parity split keeps output DMA contiguous
    y_d = nc.dram_tensor("y", [B, 2, NL // 2, C_OUT], mybir.dt.float16,
                         kind="ExternalOutput")

    with tile.TileContext(nc) as tc:
        with (
            tc.tile_pool(name="xsp", bufs=1) as xsp,
            tc.tile_pool(name="wp", bufs=3) as wp,
            tc.tile_pool(name="op", bufs=2) as op,
            tc.tile_pool(name="pp", bufs=8, space=bass.MemorySpace.PSUM) as pp,
        ):
            xt = xsp.tile([C_IN, ROWS + 2, W + 2, B], mybir.dt.float16)
            nc.sync.dma_start(out=xt[:, 0:3], in_=xs_d[0])

            for blk in range(NBLK):
                wt = wp.tile([C_IN, BL, KS * KS, C_OUT], mybir.dt.float8e3)
                nc.sync.dma_start(out=wt[:], in_=w_d[blk])
                if blk == 0:
                    nc.sync.dma_start(out=xt[:, 3:6], in_=xs_d[1])
                yt = op.tile([2 * B, NP, C_OUT], mybir.dt.float16)
                for p0 in range(0, NP, 2):
                    pss = [pp.tile([2 * B, 2 * C_OUT], mybir.dt.float32,
                                   name="ps", tag="ps") for _ in range(2)]
                    for j in range(KS * KS):
                        dh, dw = j // KS, j % KS
                        for q in range(2):
                            l0 = blk * BL + 2 * (p0 + q)
                            oh, ow = l0 // W, l0 % W
                            nc.tensor.matmul(
                                pss[q][:],
                                xt[:, oh + dh, ow + dw:ow + dw + 2, :],
                                wt[:, 2 * (p0 + q):2 * (p0 + q) + 2, j, :],
                                start=(j == 0),
                                stop=(j == KS * KS - 1),
                            )
                    for q in range(2):
                        # even location: batch partitions 0-63, cols 0-191
                        nc.vector.tensor_copy(yt[0:B, p0 + q],
                                              pss[q][0:B, 0:C_OUT])
                        # odd location: partitions 64-127, cols 192-383
                        nc.scalar.copy(yt[B:2 * B, p0 + q],
                                       pss[q][B:2 * B, C_OUT:2 * C_OUT])
                nc.sync.dma_start(
                    out=y_d[:, 0, blk * NP:(blk + 1) * NP, :],
                    in_=yt[0:B])
                nc.sync.dma_start(
                    out=y_d[:, 1, blk * NP:(blk + 1) * NP, :],
                    in_=yt[B:2 * B])

    nc.compile()
    return nc


def _prep_inputs(x, weight):
    """Host-side shard + quantize + device layout (not timed)."""
    xq = (x.astype(np.float32) / WSCALE).astype(np.float16)
    xp = np.pad(xq, ((0, 0), (0, 0), (1, 1), (1, 1)))  # [B, C, 34, 34]
    wq = np.ascontiguousarray(
        (weight.astype(np.float32) * WSCALE)
        .reshape(C_OUT, C_IN, KS * KS, NCORES, NBLK, BL)
        .transpose(3, 4, 1, 5, 2, 0)).astype(ml_dtypes.float8_e3m4)
    # wq: [core, blk, c, l, j, o]

    in_maps = []
    for c in range(NCORES):
        xs = np.ascontiguousarray(
            xp[:, :, ROWS * c:ROWS * c + ROWS + 2, :]
            .transpose(1, 2, 3, 0)
            .reshape(C_IN, 2, 3, W + 2, B)
            .transpose(1, 0, 2, 3, 4))
        in_maps.append({"xs": xs, "w": wq[c]})
    return in_maps


def kernel(x, weight, _want_trace=False, **_kw):
    global _cached
    from concourse.bass_utils import run_bass_kernel_spmd

    x = np.asarray(x)
    weight = np.asarray(weight)
    if _cached is None:
        _cached = _build_program()
    nc = _cached

    in_maps = _prep_inputs(x, weight)
    res = run_bass_kernel_spmd(nc, in_maps, list(range(NCORES)),
                               trace=_want_trace)

    y = np.empty((B, C_OUT, H, W), np.float32)
    for c in range(NCORES):
        yc = np.asarray(res.results[c]["y"]).astype(np.float32)
        # yc[b, parity, lp, o]; l = 2*lp + parity
        yl = yc.transpose(0, 2, 1, 3).reshape(B, NL, C_OUT)
        y[:, :, ROWS * c:ROWS * (c + 1), :] = (
            yl.reshape(B, ROWS, W, C_OUT).transpose(0, 3, 1, 2))
    if _want_trace:
        return y, res
    return y


# revision 4
# speedup vs baseline: 1.2855x; 1.2855x over previous
"""LocallyConnected2d Trainium2 kernel (v4).

y[b,o,l] = sum_k x_unf[b,k,l] * w[o,k,l]   (B=64, K=864, L=1024, O=192)

Strategy: shard L across 8 cores (128 locations = 4 output rows each).

Traffic: weights dominate (read once, 170M elements). Stored fp8e3
(4 mantissa bits; w is uniform so e3m4 with scale 320 gives ~1.2% rms
quant err -> ~1.1e-2 max rel output err). The 1/320 descale is folded
into x on the host. x is NOT unfolded: the raw padded slice
[96c, 6h, 34w, 64b] fp16 lives in SBUF and every matmul's stationary
operand is a shifted window of it (zero-copy unfold).

Compute: contraction ordered (kh,kw)-major -> 9 chunks of K=96 (=C_IN).
The PE runs in 128x64 column-tiled mode: even locations compute on
tile T0 (PSUM partitions 0-63), odd locations on T1 (64-127), so two
locations' matmul streams execute concurrently. Stationary = x window
[96, 64], moving = per-location weights [96, 192] fp8e3 at 1 row/cyc.
bass derives tile_position from the PSUM slice's base partition.

DMA: SDMA engines stream ~22 GB/s with <=3.5KB descriptors but only
~14 GB/s with >=6KB ones, so the w stream is issued as 4 dma_starts
per 8-location block (3456B descriptors) and x as half-width slices
(2176B). y evacuates via vector (even) / scalar (odd) PSUM copies
into a parity-compacted fp16 staging tile whose output DMA (scalar
queue, to offload sync) reads contiguous 1.5KB lines.
"""

import sys

sys.path.insert(0, "/opt/trn_rl_repo")

import numpy as np
import ml_dtypes

B = 64
C_IN = 96
H = W = 32
C_OUT = 192
KS = 3
L = 1024
NCORES = 8
NL = L // NCORES      # 128 locations per core
ROWS = H // NCORES    # 4 output rows per core
NBLK = 16             # blocks of 8 locations
BL = 8                # locations per block
NP = BL // 2          # location pairs (even/odd) per block
WSCALE = 320.0        # fp8e3 weight scale (max |w|*320 ~ 15.4 < 15.5)

_cached = None


def _build_program():
    from concourse import bacc, bass, tile, mybir

    nc = bacc.Bacc("TRN2", target_bir_lowering=False, debug=False,
                   num_devices=NCORES)
    # x split in two 3-row chunks so compute can start after chunk 0
    xs_d = nc.dram_tensor("xs", [2, C_IN, 3, W + 2, B], mybir.dt.float16,
                          kind="ExternalInput")
    w_d = nc.dram_tensor("w", [NBLK, C_IN, BL, KS * KS, C_OUT],
                         mybir.dt.float8e3, kind="ExternalInput")
    # y[b, parity, l//2, o]; parity split keeps output DMA contiguous
    y_d = nc.dram_tensor("y", [B, 2, NL // 2, C_OUT], mybir.dt.float16,
                         kind="ExternalOutput")

    HW = (W + 2) // 2  # half-width x DMA slices -> 2176B descriptors

    with tile.TileContext(nc) as tc:
        with (
            tc.tile_pool(name="xsp", bufs=1) as xsp,
            tc.tile_pool(name="wp", bufs=3) as wp,
            tc.tile_pool(name="op", bufs=2) as op,
            tc.tile_pool(name="pp", bufs=8, space=bass.MemorySpace.PSUM) as pp,
        ):
            xt = xsp.tile([C_IN, ROWS + 2, W + 2, B], mybir.dt.float16)
            nc.sync.dma_start(out=xt[:, 0:3, 0:HW], in_=xs_d[0, :, :, 0:HW])
            nc.sync.dma_start(out=xt[:, 0:3, HW:W + 2],
                              in_=xs_d[0, :, :, HW:W + 2])

            for blk in range(NBLK):
                wt = wp.tile([C_IN, BL, KS * KS, C_OUT], mybir.dt.float8e3)
                for i in range(0, BL, 2):
                    nc.sync.dma_start(out=wt[:, i:i + 2],
                                      in_=w_d[blk, :, i:i + 2])
                if blk == 0:
                    nc.sync.dma_start(out=xt[:, 3:6, 0:HW],
                                      in_=xs_d[1, :, :, 0:HW])
                    nc.sync.dma_start(out=xt[:, 3:6, HW:W + 2],
                                      in_=xs_d[1, :, :, HW:W + 2])
                yt = op.tile([2 * B, NP, C_OUT], mybir.dt.float16)
                for p in range(NP):
                    l0 = blk * BL + 2 * p
                    oh, ow = l0 // W, l0 % W
                    ps = pp.tile([2 * B, C_OUT], mybir.dt.float32,
                                 name="ps", tag="ps")
                    for j in range(KS * KS):
                        dh, dw = j // KS, j % KS
                        # even location -> PE column tile T0 (PSUM 0-63)
                        nc.tensor.matmul(
                            ps[0:B],
                            xt[:, oh + dh, ow + dw, :],
                            wt[:, 2 * p, j, :],
                            start=(j == 0),
                            stop=(j == KS * KS - 1),
                        )
                        # odd location -> PE column tile T1 (PSUM 64-127)
                        nc.tensor.matmul(
                            ps[B:2 * B],
                            xt[:, oh + dh, ow + dw + 1, :],
                            wt[:, 2 * p + 1, j, :],
                            start=(j == 0),
                            stop=(j == KS * KS - 1),
                        )
                    nc.vector.tensor_copy(yt[0:B, p], ps[0:B])
                    nc.scalar.copy(yt[B:2 * B, p], ps[B:2 * B])
                nc.scalar.dma_start(
                    out=y_d[:, 0, blk * NP:(blk + 1) * NP, :],
                    in_=yt[0:B])
                nc.scalar.dma_start(
                    out=y_d[:, 1, blk * NP:(blk + 1) * NP, :],
                    in_=yt[B:2 * B])

    nc.compile()
    return nc


def _prep_inputs(x, weight):
    """Host-side shard + quantize + device layout (not timed)."""
    xq = (x.astype(np.float32) / WSCALE).astype(np.float16)
    xp = np.pad(xq, ((0, 0), (0, 0), (1, 1), (1, 1)))  # [B, C, 34, 34]
    wq = np.ascontiguousarray(
        (weight.astype(np.float32) * WSCALE)
        .reshape(C_OUT, C_IN, KS * KS, NCORES, NBLK, BL)
        .transpose(3, 4, 1, 5, 2, 0)).astype(ml_dtypes.float8_e3m4)
    # wq: [core, blk, c, l, j, o]

    in_maps = []
    for c in range(NCORES):
        xs = np.ascontiguousarray(
            xp[:, :, ROWS * c:ROWS * c + ROWS + 2, :]
            .transpose(1, 2, 3, 0)
            .reshape(C_IN, 2, 3, W + 2, B)
            .transpose(1, 0, 2, 3, 4))
        in_maps.append({"xs": xs, "w": wq[c]})
    return in_maps


def kernel(x, weight, _want_trace=False, **_kw):
    global _cached
    from concourse.bass_utils import run_bass_kernel_spmd

    x = np.asarray(x)
    weight = np.asarray(weight)
    if _cached is None:
        _cached = _build_program()
    nc = _cached

    in_maps = _prep_inputs(x, weight)
    res = run_bass_kernel_spmd(nc, in_maps, list(range(NCORES)),
                               trace=_want_trace)

    y = np.empty((B, C_OUT, H, W), np.float32)
    for c in range(NCORES):
        yc = np.asarray(res.results[c]["y"]).astype(np.float32)
        # yc[b, parity, lp, o]; l = 2*lp + parity
        yl = yc.transpose(0, 2, 1, 3).reshape(B, NL, C_OUT)
        y[:, :, ROWS * c:ROWS * (c + 1), :] = (
            yl.reshape(B, ROWS, W, C_OUT).transpose(0, 3, 1, 2))
    if _want_trace:
        return y, res
    return y
